# revision 1
# baseline (speedup 1.0000x reference)
"""Trainium2 Bass kernel for nn_DihedralAngleLayer.

Input:  x [2_000_000, 42] f32 (14 atoms x 3 coords per row),
        mask_matrix [4, 14] f32 one-hot carbon selector.
Output: dihedral angle per row, [2_000_000] f32.

Data-parallel across 8 NeuronCores: rows are padded to 8*250_112 and split
evenly. Each core owns rows in global partition-major order: partition p
handles rows [p*Q, (p+1)*Q), Q = rows/128. Per tile (G columns of every
partition) the Vector engine computes

    a = c0-c1, b = c2-c1, d = c3-c2, m = b x d
    r=a.b p=a.d det=a.m q=b.b s=b.d       (dup-write + shifted-AP crosses)
    xx = p*q - r*s        # Lagrange identity for (a x b).(d x b)
    yy = sqrt(q)*det      # |v1| * scalar triple product

writing xx,yy into full-length planes; the per-row-scalar atan2 tail
(range-reduced arctan on ScalarE) runs on multi-tile chunks so the ~0.5us
per-instruction floor amortizes. dm2/da1 run on GPSIMD to offload DVE.
"""

import numpy as np

import concourse.bacc as bacc
import concourse.bass as bass
import concourse.mybir as mybir
from concourse.bass_utils import run_bass_kernel_spmd
from concourse.tile import TileContext

AF = mybir.ActivationFunctionType
OP = mybir.AluOpType
F32 = mybir.dt.float32

PI = float(np.pi)

N_CORES = 8
G_TILE = 192
# first two tiles halved so DVE starts ~4us sooner (smaller first DMA);
# trailing 34-tile gets its own chunk so almost no tail work is exposed
# after the last head finishes.
TILES = [96, 96] + [G_TILE] * 9 + [34]   # sum = 1954
CHUNK_AFTER = {3, 7, 10, 11}             # tile indices closing a tail chunk
Q = sum(TILES)                      # rows per partition
ROWS_PER_CORE = 128 * Q            # 250_112
TILES_PER_CHUNK = 4

# row-interleaved scratch layout per row-group (period 39 floats)
PER = 39
S_A, S_B2, S_D2, S_M = 0, 3, 9, 15
P_1, P_2 = 18, 21
D_0 = 24
# per-tile mini-planes ([G] each) for dots + pq/rs/sq: r,p,det,q,s,pq,rs,sq
RP_R, RP_P, RP_DET, RP_Q, RP_S, RP_PQ, RP_RS, RP_SQ = range(8)

# chunk-tail scratch: 8 slots of CS_FD floats each (slots reused over the chain)
CS_FD = G_TILE * TILES_PER_CHUNK


def _ap(base, off, dims):
    return bass.AP(
        base.tensor, base.offset + off, [list(base.ap[0])] + [list(d) for d in dims]
    )


def _emit_head(nc, xp, scp, rp, x, xyf, toff, G, c0, c1, c2, c3):
    """Per-tile head: subs, cross, dots, xx/yy -> full-length planes."""
    v, s, g = nc.vector, nc.scalar, nc.gpsimd

    xt = xp.tile([128, G * 42], F32, tag="x")
    sc = scp.tile([128, G * PER], F32, tag="sc")
    r5 = rp.tile([128, G * 8], F32, tag="r5")

    nc.gpsimd.dma_start(
        out=xt[:],
        in_=x.rearrange("(p q) c -> p q c", p=128)[:, toff : toff + G, :],
    )

    xa, sa, ra = xt[:], sc[:], r5[:]

    def xap(off, dims):
        return _ap(xa, off, [[42, G]] + dims)

    def sap(off, dims=()):
        return _ap(sa, off, [[PER, G]] + list(dims))

    def rap(k, dims=None):
        return _ap(ra, k * G, dims if dims is not None else [[1, G]])

    # a = c0-c1
    v.tensor_tensor(sap(S_A, [[1, 3]]), xap(c0, [[1, 3]]), xap(c1, [[1, 3]]), OP.subtract)
    # duplicated b = c2-c1 and d = c3-c2 (ISA allows max 3 free dims per AP)
    v.tensor_tensor(
        sap(S_B2, [[3, 2], [1, 3]]),
        xap(c2, [[0, 2], [1, 3]]),
        xap(c1, [[0, 2], [1, 3]]),
        OP.subtract,
    )
    v.tensor_tensor(
        sap(S_D2, [[3, 2], [1, 3]]),
        xap(c3, [[0, 2], [1, 3]]),
        xap(c2, [[0, 2], [1, 3]]),
        OP.subtract,
    )
    # P1 = b_yzx*d_zxy ; P2 = b_zxy*d_yzx
    v.tensor_tensor(
        sap(P_1, [[3, 2], [1, 3]]),
        sap(S_B2 + 1, [[1, 2], [1, 3]]),
        sap(S_D2 + 2, [[-1, 2], [1, 3]]),
        OP.mult,
    )
    # m = P1 - P2
    v.tensor_tensor(sap(S_M, [[1, 3]]), sap(P_1, [[1, 3]]), sap(P_2, [[1, 3]]), OP.subtract)
    # three-prods of a with {b,d,m} -> rt,pt,dett   (DVE)
    v.tensor_tensor(
        sap(D_0, [[3, 3], [1, 3]]),
        sap(S_A, [[0, 3], [1, 3]]),
        sap(S_B2, [[6, 3], [1, 3]]),
        OP.mult,
    )
    # qt = b*b on ScalarE (Square is a filler in every ACT table set, and the
    # Scalar engine is far from saturated); st = b*d stays on DVE.
    # (GPSIMD tensor work is a net loss: it contends with DVE on the shared
    #  SBUF port and inflates every concurrent DVE op up to 2.4x — measured.)
    s.activation(sap(D_0 + 9, [[1, 3]]), sap(S_B2, [[1, 3]]), AF.Square)
    v.tensor_tensor(
        sap(D_0 + 12, [[1, 3]]),
        sap(S_B2, [[1, 3]]),
        sap(S_D2, [[1, 3]]),
        OP.mult,
    )
    # segmented reduce -> dots r,p,det,q,s as per-tile mini-planes: iterating
    # (dot, row, comp) makes both the reads and the plane writes unit-inner
    v.reduce_sum(
        rap(RP_R, [[G, 5], [1, G]]),
        _ap(sa, D_0, [[3, 5], [PER, G], [1, 3]]),
        axis=mybir.AxisListType.X,
    )
    # [pq, rs] on planes
    v.tensor_tensor(
        rap(RP_PQ, [[G, 2], [1, G]]),
        rap(RP_P, [[-G, 2], [1, G]]),
        rap(RP_Q, [[G, 2], [1, G]]),
        OP.mult,
    )
    # xx -> full plane (all unit stride)
    v.tensor_tensor(_ap(xyf, toff, [[1, G]]), rap(RP_PQ), rap(RP_RS), OP.subtract)
    # sq = sqrt(q); yy = sq*det -> full plane
    s.activation(rap(RP_SQ), rap(RP_Q), AF.Sqrt)
    v.tensor_tensor(_ap(xyf, Q + toff, [[1, G]]), rap(RP_SQ), rap(RP_DET), OP.mult)


def _emit_tail(nc, csp, outp, y, xyf, toff, FD):
    """Chunk tail: atan2 on [128, FD] contiguous planes."""
    v, s = nc.vector, nc.scalar

    cs = csp.tile([128, 7 * CS_FD], F32, tag="cs")
    ot = outp.tile([128, CS_FD], F32, tag="o")
    ca = cs[:]

    def cap(k, n=1):
        return _ap(ca, k * CS_FD, [[1, FD]] if n == 1 else [[CS_FD, n], [1, FD]])

    def xy(n=1):
        return _ap(xyf, toff, [[Q, n], [1, FD]] if n > 1 else [[1, FD]])

    # slots: 0:ax/e2  1:ay/sy  2:df/u  3:mn/v  4:mx/rq  5:rmx/al  6:e1/z
    s.activation(cap(0, 2), xy(2), AF.Abs)                       # ax,ay
    v.tensor_tensor(cap(2), cap(0), cap(1), OP.subtract)          # df
    v.tensor_tensor(cap(3), cap(0), cap(1), OP.min)               # mn
    v.tensor_tensor(cap(4), cap(0), cap(1), OP.max)               # mx
    v.reciprocal_approx_fast(cap(5), cap(4))                      # rmx
    v.tensor_tensor(cap(4), cap(3), cap(5), OP.mult)              # rq (mx slot)
    s.activation(cap(5), cap(4), AF.Arctan)                       # al (rmx slot)
    s.activation(cap(6), cap(2), AF.Sign)                         # e1
    s.activation(cap(0, 2), xy(2), AF.Sign)                       # e2,sy (ax/ay slots)
    v.tensor_tensor(cap(2), cap(6), cap(0), OP.mult)              # u (df slot)
    v.tensor_tensor(cap(3), cap(5), cap(2), OP.mult)              # v (mn slot)
    v.scalar_tensor_tensor(cap(5), cap(2), PI / 4, cap(3), OP.mult, OP.subtract)  # w2
    v.scalar_tensor_tensor(cap(6), cap(0), PI / 4, cap(5), OP.mult, OP.add)       # z
    v.scalar_tensor_tensor(
        _ap(ot[:], 0, [[1, FD]]), cap(6), PI / 2, cap(1), OP.subtract, OP.mult
    )
    nc.gpsimd.dma_start(
        out=y.rearrange("(p q) -> p q", p=128)[:, toff : toff + FD],
        in_=_ap(ot[:], 0, [[1, FD]]),
    )


def build_kernel(atoms):
    c0, c1, c2, c3 = (3 * int(a) for a in atoms)
    nc = bacc.Bacc("TRN2", target_bir_lowering=False, debug=False)
    x = nc.dram_tensor("x", [ROWS_PER_CORE, 42], F32, kind="ExternalInput")
    y = nc.dram_tensor("y", [ROWS_PER_CORE], F32, kind="ExternalOutput")
    with TileContext(nc) as tc:
        with (
            tc.tile_pool(name="xp", bufs=2) as xp,
            tc.tile_pool(name="scp", bufs=2) as scp,
            tc.tile_pool(name="rp", bufs=2) as rp,
            tc.tile_pool(name="xyp", bufs=1) as xyp,
            tc.tile_pool(name="csp", bufs=1) as csp,
            tc.tile_pool(name="outp", bufs=2) as outp,
        ):
            xyf_tile = xyp.tile([128, 2 * Q], F32, tag="xy")
            xyf = xyf_tile[:]
            toff = 0
            chunk_start = 0
            for i, G in enumerate(TILES):
                _emit_head(nc, xp, scp, rp, x, xyf, toff, G, c0, c1, c2, c3)
                toff += G
                if i in CHUNK_AFTER or i == len(TILES) - 1:
                    _emit_tail(nc, csp, outp, y, xyf, chunk_start, toff - chunk_start)
                    chunk_start = toff
    nc.finalize()
    return nc


_CACHE = {}


def _get_nc(atoms):
    key = tuple(int(a) for a in atoms)
    if key not in _CACHE:
        _CACHE[key] = build_kernel(key)
    return _CACHE[key]


def run(x, atoms=(0, 4, 7, 11), **spmd_kwargs):
    """x: [B, 42] f32. Returns (y [B] f32, BassKernelResults)."""
    x = np.ascontiguousarray(np.asarray(x, dtype=np.float32))
    B = x.shape[0]
    total = N_CORES * ROWS_PER_CORE
    if B < total:
        # pad with replicated leading rows (valid, non-degenerate data)
        x = np.concatenate([x, x[: total - B]], axis=0)
    nc = _get_nc(atoms)
    shards = x.reshape(N_CORES, ROWS_PER_CORE, 42)
    in_maps = [{"x": shards[i]} for i in range(N_CORES)]
    res = run_bass_kernel_spmd(nc, in_maps, core_ids=list(range(N_CORES)), **spmd_kwargs)
    y = np.concatenate([r["y"] for r in res.results])[:B]
    return np.asarray(y, dtype=np.float32), res


def kernel(x, mask_matrix):
    mask = np.asarray(mask_matrix)
    atoms = tuple(int(i) for i in np.argmax(mask, axis=1))
    y, _ = run(x, atoms=atoms)
    return y

